# revision 3
# baseline (speedup 1.0000x reference)
import sys

sys.path.insert(0, "/opt/trn_rl_repo")

import numpy as np
import ml_dtypes

from concourse import bass, bacc, mybir
from concourse import tile
from concourse.bass_utils import run_bass_kernel_spmd

BF16 = mybir.dt.bfloat16
F32 = mybir.dt.float32
NPBF16 = ml_dtypes.bfloat16

B, C, H, W = 16, 256, 1, 4096
NCORES = 8
BL = B // NCORES          # batches per core
NBLK = W // 128           # 32 w-blocks of 128
BW = 130                  # band width per block (128 + 2 halo output cols)


def _build_graph():
    """One NeuronCore graph (SPMD across 8 cores).

    Per batch b (2 per core), x_b = [C=256, W=4096] bf16 (CW layout):
      Y = M'^T x (CW, padded by one zero col each side), M' = Wq^T Wk / 16
      V = Wv x computed directly in WC orientation (x chunks as lhsT)
      per w-block i: gram G[p=w', j] = sum_c x[c, i*128+p] * Y[c, i*128-1+j]
      E = exp(G) * bandmask;  denom = colsum(E) via ones-matmul (replicated
      over partitions);  stitch cross-block denom pieces;  recip;  En = E*recip
      O_raw[c, j] = sum_p V[i*128+p, c] * En[p, j];  stitch + assemble; DMA out.
    """
    nc = bacc.Bacc(None, target_bir_lowering=False, debug=False)

    x_d = nc.declare_dram_parameter("x", [BL, 2, 128, W], BF16, isOutput=False)
    mp_d = nc.declare_dram_parameter("Mp", [2, 128, 256], BF16, isOutput=False)
    wv_d = nc.declare_dram_parameter("WvT", [2, 128, 256], BF16, isOutput=False)
    ub_d = nc.declare_dram_parameter("ub", [2, 128, 1], F32, isOutput=False)
    msk_d = nc.declare_dram_parameter("mask", [128, BW], BF16, isOutput=False)
    ones_d = nc.declare_dram_parameter("onesm", [128, 128], BF16, isOutput=False)
    out_d = nc.declare_dram_parameter("out", [BL, 2, 128, W], F32, isOutput=True)

    Exp = mybir.ActivationFunctionType.Exp
    Identity = mybir.ActivationFunctionType.Identity
    MUL = mybir.AluOpType.mult
    ADD = mybir.AluOpType.add

    with tile.TileContext(nc) as tc:
        with (
            tc.tile_pool(name="const", bufs=1) as cpool,
            tc.tile_pool(name="xin", bufs=1) as xpool,
            tc.tile_pool(name="ybuf", bufs=1) as ypool,
            tc.tile_pool(name="vbuf", bufs=1) as vpool,
            tc.tile_pool(name="ebuf", bufs=1) as epool,
            tc.tile_pool(name="escr", bufs=4) as espool,
            tc.tile_pool(name="den", bufs=1) as dpool,
            tc.tile_pool(name="oraw", bufs=1) as orpool,
            tc.tile_pool(name="ofin", bufs=1) as ofpool,
            tc.tile_pool(name="mm2k", bufs=2, space=bass.MemorySpace.PSUM) as mmpool,
            tc.tile_pool(name="vps", bufs=2, space=bass.MemorySpace.PSUM) as vppool,
            tc.tile_pool(name="gps", bufs=2, space=bass.MemorySpace.PSUM) as gppool,
            tc.tile_pool(name="dps", bufs=2, space=bass.MemorySpace.PSUM) as dppool,
        ):
            # ---- constants ----
            mp_sb = cpool.tile([128, 2, 256], BF16, tag="mp")
            wv_sb = cpool.tile([128, 2, 256], BF16, tag="wv")
            ub_sb = cpool.tile([128, 2, 1], F32, tag="ub")
            msk_sb = cpool.tile([128, BW], BF16, tag="msk")
            ones_sb = cpool.tile([128, 128], BF16, tag="ones")
            for ch in range(2):
                nc.sync.dma_start(mp_sb[:, ch, :], mp_d[ch])
                nc.sync.dma_start(wv_sb[:, ch, :], wv_d[ch])
                nc.sync.dma_start(ub_sb[:, ch, :], ub_d[ch])
            nc.sync.dma_start(msk_sb[:], msk_d[:])
            nc.sync.dma_start(ones_sb[:], ones_d[:])

            for b in range(BL):
                # ---- load x (bf16, host pre-cast) ----
                x_sb = xpool.tile([128, 2, W], BF16, tag="x")
                for ch in range(2):
                    nc.sync.dma_start(x_sb[:, ch, :], x_d[b, ch])

                # ---- Y projection (CW), zero-padded one col each side ----
                y_sb = ypool.tile([128, 2, W + 2], BF16, tag="y")
                for ch in range(2):
                    nc.vector.memset(y_sb[:, ch, 0:1], 0.0)
                    nc.vector.memset(y_sb[:, ch, W + 1 : W + 2], 0.0)
                for mch in range(2):
                    for n in range(8):
                        yp = mmpool.tile([128, 512], F32, tag="mm")
                        for kch in range(2):
                            nc.tensor.matmul(
                                yp[:],
                                mp_sb[:, kch, mch * 128 : (mch + 1) * 128],
                                x_sb[:, kch, n * 512 : (n + 1) * 512],
                                start=(kch == 0),
                                stop=(kch == 1),
                            )
                        nc.scalar.activation(
                            y_sb[:, mch, 1 + n * 512 : 1 + (n + 1) * 512],
                            yp[:],
                            Identity,
                            bias=ub_sb[:, mch, :],
                        )

                # ---- per-block: V proj (WC), gram band, exp*mask, denom ----
                v_sb = vpool.tile([128, NBLK, 256], BF16, tag="v")
                e_sb = epool.tile([128, NBLK, BW], BF16, tag="e")
                den_sb = dpool.tile([128, NBLK, BW], F32, tag="d")
                for i in range(NBLK):
                    vp = vppool.tile([128, 256], F32, tag="vp")
                    gp = gppool.tile([128, BW], F32, tag="gp")
                    for kch in range(2):
                        xchunk = x_sb[:, kch, i * 128 : (i + 1) * 128]
                        nc.tensor.matmul(
                            vp[:], xchunk, wv_sb[:, kch, :],
                            start=(kch == 0), stop=(kch == 1),
                        )
                        nc.tensor.matmul(
                            gp[:], xchunk, y_sb[:, kch, i * 128 : i * 128 + BW],
                            start=(kch == 0), stop=(kch == 1),
                        )
                    if i % 2 == 0:
                        nc.scalar.copy(v_sb[:, i, :], vp[:])
                    else:
                        nc.vector.tensor_copy(v_sb[:, i, :], vp[:])
                    es = espool.tile([128, BW], BF16, tag="es")
                    nc.scalar.activation(es[:], gp[:], Exp)
                    nc.vector.tensor_tensor(e_sb[:, i, :], es[:], msk_sb[:], op=MUL)
                    dp = dppool.tile([128, BW], F32, tag="dp")
                    nc.tensor.matmul(dp[:], ones_sb[:], e_sb[:, i, :],
                                     start=True, stop=True)
                    nc.vector.tensor_copy(den_sb[:, i, :], dp[:])

                # ---- stitch denom pieces -> denfull [128, W+2] (replicated) ----
                denf = dpool.tile([128, W + 2], F32, tag="df")
                nc.vector.memset(denf[:, 0:1], 1.0)
                nc.vector.memset(denf[:, W + 1 : W + 2], 1.0)
                dmain = denf[:, 1 : W + 1].rearrange("p (i j) -> p i j", j=128)
                nc.vector.tensor_copy(dmain, den_sb[:, :, 1:129])
                # w = i*128+127 needs block i+1 piece col 0  (i = 0..30)
                nc.vector.tensor_tensor(
                    denf[:, 128 : W : 128],
                    denf[:, 128 : W : 128],
                    den_sb[:, 1:32, 0],
                    op=ADD,
                )
                # w = i*128 needs block i-1 piece col 129  (i = 1..31)
                nc.vector.tensor_tensor(
                    denf[:, 129 : W : 128],
                    denf[:, 129 : W : 128],
                    den_sb[:, 0:31, 129],
                    op=ADD,
                )
                recip = dpool.tile([128, W + 2], F32, tag="rc")
                nc.vector.reciprocal_approx_fast(recip[:], denf[:])

                # ---- normalize E, combine with V, copy out of PSUM ----
                or_sb = orpool.tile([128, 2, NBLK, BW], BF16, tag="or")
                for i in range(NBLK):
                    en = espool.tile([128, BW], BF16, tag="en")
                    nc.vector.tensor_tensor(
                        en[:], e_sb[:, i, :], recip[:, i * 128 : i * 128 + BW], op=MUL
                    )
                    for cch in range(2):
                        op_ = mmpool.tile([128, 512], F32, tag="mm")
                        nc.tensor.matmul(
                            op_[:, 0:BW],
                            v_sb[:, i, cch * 128 : (cch + 1) * 128],
                            en[:],
                            start=True, stop=True,
                        )
                        if cch == 0:
                            nc.scalar.copy(or_sb[:, cch, i, :], op_[:, 0:BW])
                        else:
                            nc.vector.tensor_copy(or_sb[:, cch, i, :], op_[:, 0:BW])

                # ---- assemble final output + stitch block-edge partials ----
                of_sb = ofpool.tile([128, 2, W], BF16, tag="of")
                for cch in range(2):
                    nc.vector.tensor_copy(
                        of_sb[:, cch, :].rearrange("p (i j) -> p i j", j=128),
                        or_sb[:, cch, :, 1:129],
                    )
                    nc.vector.tensor_tensor(
                        of_sb[:, cch, 127 : W - 1 : 128],
                        of_sb[:, cch, 127 : W - 1 : 128],
                        or_sb[:, cch, 1:32, 0],
                        op=ADD,
                    )
                    nc.vector.tensor_tensor(
                        of_sb[:, cch, 128 : W : 128],
                        of_sb[:, cch, 128 : W : 128],
                        or_sb[:, cch, 0:31, 129],
                        op=ADD,
                    )
                    nc.gpsimd.dma_start(out_d[b, cch], of_sb[:, cch, :])

    nc.compile()
    return nc


_GRAPH = None


def kernel(x, Wq, bq, Wk, bk, Wv, bv):
    global _GRAPH
    x = np.asarray(x, np.float32)
    Wq = np.asarray(Wq, np.float32)
    Wk = np.asarray(Wk, np.float32)
    Wv = np.asarray(Wv, np.float32)
    bq = np.asarray(bq, np.float32)
    bk = np.asarray(bk, np.float32)
    bv = np.asarray(bv, np.float32)

    Mp = (Wq.T @ Wk) / 16.0                       # M'[c, c']
    ub = (Wk.T @ bq) / 16.0                       # per-c' bias on Y
    mask = np.zeros((128, BW), np.float32)
    for p in range(128):
        mask[p, p : p + 3] = 1.0
    onesm = np.ones((128, 128), np.float32)

    xs = x[:, :, 0, :]                            # [B, C, W]
    in_maps = []
    for core in range(NCORES):
        shard = xs[core * BL : (core + 1) * BL].reshape(BL, 2, 128, W)
        in_maps.append({
            "x": shard.astype(NPBF16),
            "Mp": Mp.reshape(2, 128, 256).astype(NPBF16),
            "WvT": Wv.T.reshape(2, 128, 256).astype(NPBF16),
            "ub": ub.reshape(2, 128, 1).astype(np.float32),
            "mask": mask.astype(NPBF16),
            "onesm": onesm.astype(NPBF16),
        })

    if _GRAPH is None:
        _GRAPH = _build_graph()
    res = run_bass_kernel_spmd(_GRAPH, in_maps, core_ids=list(range(NCORES)))
    outs = [np.asarray(r["out"], np.float32).reshape(BL, C, W) for r in res.results]
    full = np.concatenate(outs, axis=0)           # [B, C, W]
    full = full + bv[None, :, None]               # bias on V folds through softmax
    return full[:, :, None, :].astype(np.float32)


# revision 24
# speedup vs baseline: 466.4047x; 466.4047x over previous
import sys

sys.path.insert(0, "/opt/trn_rl_repo")

import numpy as np
import ml_dtypes

from concourse import bass, bacc, mybir
from concourse import tile
from concourse.bass_utils import run_bass_kernel_spmd

BF16 = mybir.dt.bfloat16
F32 = mybir.dt.float32
NPBF16 = ml_dtypes.bfloat16

B, C, H, W = 16, 256, 1, 4096
NCORES = 8
BL = B // NCORES          # batches per core
NBLK = W // 128           # 32 w-blocks of 128
BW = 130                  # band width per block (128 + 2 halo output cols)


def _build_graph(with_bias=False):
    """One NeuronCore graph (SPMD across 8 cores).

    Per batch b (2 per core), x_b = [C=256, W=4096] bf16 (CW layout):
      Y = M'^T x (CW, padded by one zero col each side), M' = Wq^T Wk / 16
      V = Wv x computed directly in WC orientation (x chunks as lhsT)
      per w-block i: gram G[p=w', j] = sum_c x[c, i*128+p] * Y[c, i*128-1+j]
      E = exp(G) * bandmask;  denom = colsum(E) via ones-matmul (replicated
      over partitions);  stitch cross-block denom pieces;  recip;  En = E*recip
      O_raw[c, j] = sum_p V[i*128+p, c] * En[p, j];  stitch + assemble; DMA out.
    """
    nc = bacc.Bacc(None, target_bir_lowering=False, debug=False)

    x_d = nc.declare_dram_parameter("x", [BL, 2, 128, W], BF16, isOutput=False)
    mp_d = nc.declare_dram_parameter("Mp", [2, 128, 256], BF16, isOutput=False)
    wv_d = nc.declare_dram_parameter("WvT", [2, 128, 256], BF16, isOutput=False)
    ub_d = nc.declare_dram_parameter("ub", [2, 128, 1], F32, isOutput=False)
    msk_d = nc.declare_dram_parameter("mask", [128, BW], BF16, isOutput=False)
    ones_d = nc.declare_dram_parameter("onesm", [128, 128], BF16, isOutput=False)
    out_d = nc.declare_dram_parameter("out", [BL, 2, 128, W], F32, isOutput=True)

    Exp = mybir.ActivationFunctionType.Exp
    Identity = mybir.ActivationFunctionType.Identity
    MUL = mybir.AluOpType.mult
    ADD = mybir.AluOpType.add

    with tile.TileContext(nc) as tc:
        with (
            tc.tile_pool(name="const", bufs=1) as cpool,
            tc.tile_pool(name="xin", bufs=2) as xpool,
            tc.tile_pool(name="ybuf", bufs=2) as ypool,
            tc.tile_pool(name="vbuf", bufs=2) as vpool,
            tc.tile_pool(name="ebuf", bufs=2) as epool,
            tc.tile_pool(name="escr", bufs=6) as espool,
            tc.tile_pool(name="den", bufs=2) as dpool,
            tc.tile_pool(name="edg", bufs=3) as egpool,
            tc.tile_pool(name="ofin", bufs=1) as ofpool,
            tc.tile_pool(name="mm2k", bufs=3, space=bass.MemorySpace.PSUM) as mmpool,
            tc.tile_pool(name="vps", bufs=2, space=bass.MemorySpace.PSUM) as vppool,
            tc.tile_pool(name="gps", bufs=3, space=bass.MemorySpace.PSUM) as gppool,
        ):
            # ---- constants ----
            mp_sb = cpool.tile([128, 2, 256], BF16, tag="mp")
            wv_sb = cpool.tile([128, 2, 256], BF16, tag="wv")
            ub_sb = cpool.tile([128, 2, 1], F32, tag="ub")
            msk_sb = cpool.tile([128, 3, BW], BF16, tag="msk")
            ones_sb = cpool.tile([128, 128], BF16, tag="ones")
            for ch in range(2):
                nc.scalar.dma_start(mp_sb[:, ch, :], mp_d[ch])
                nc.scalar.dma_start(wv_sb[:, ch, :], wv_d[ch])
                nc.scalar.dma_start(ub_sb[:, ch, :], ub_d[ch])
            for t in range(3):
                nc.scalar.dma_start(msk_sb[:, t, :], msk_d[:])
            nc.scalar.dma_start(ones_sb[:], ones_d[:])

            st = [dict() for _ in range(BL)]

            def phase_load_y(b):
                s = st[b]
                x_sb = xpool.tile([128, 2, W], BF16, tag="x", name=f"x_sb{b}")
                s["x"] = x_sb
                for ch in range(2):
                    for hf in range(2):
                        nc.sync.dma_start(
                            x_sb[:, ch, hf * (W // 2) : (hf + 1) * (W // 2)],
                            x_d[b, ch][:, hf * (W // 2) : (hf + 1) * (W // 2)],
                        )
                y_sb = ypool.tile([128, 2, W + 2], BF16, tag="y", name=f"y_sb{b}")
                s["y"] = y_sb
                for ch in range(2):
                    nc.vector.memset(y_sb[:, ch, 0:1], 0.0)
                    nc.vector.memset(y_sb[:, ch, W + 1 : W + 2], 0.0)
                for n in range(8):
                    for mch in range(2):
                        yp = mmpool.tile([128, 512], F32, tag="mm",
                                         name=f"yp{b}_{n}_{mch}")
                        for kch in range(2):
                            nc.tensor.matmul(
                                yp[:],
                                mp_sb[:, kch, mch * 128 : (mch + 1) * 128],
                                x_sb[:, kch, n * 512 : (n + 1) * 512],
                                start=(kch == 0),
                                stop=(kch == 1),
                            )
                        ydst = y_sb[:, mch, 1 + n * 512 : 1 + (n + 1) * 512]
                        if with_bias:
                            if mch == 0:
                                nc.scalar.activation(
                                    ydst, yp[:], Identity, bias=ub_sb[:, mch, :])
                            else:
                                nc.vector.tensor_scalar_add(
                                    ydst, yp[:], ub_sb[:, mch, :])
                        else:
                            if mch == 0:
                                nc.scalar.copy(ydst, yp[:])
                            else:
                                nc.vector.tensor_copy(ydst, yp[:])

            def phase_blocks(b):
                s = st[b]
                x_sb, y_sb = s["x"], s["y"]
                denf = dpool.tile([128, W + 2], F32, tag="df", name=f"denf{b}")
                dedge = egpool.tile([128, NBLK, 2], F32, tag="de", name=f"dedge{b}")
                recip = denf
                s["denf"], s["dedge"], s["recip"] = denf, dedge, recip
                nc.vector.memset(denf[:, 0:1], 1.0)
                nc.vector.memset(denf[:, W + 1 : W + 2], 1.0)

                def den_group(t):
                    lo = 3 * t
                    hi = min(lo + 3, NBLK)
                    nb = hi - lo
                    ng = nb * BW
                    dp = gppool.tile([128, 3 * BW], F32, tag="gp", name=f"dp{b}_{t}")
                    nc.tensor.matmul(dp[:, 0:ng], ones_sb[:], e_sb[:, lo:hi, :],
                                     start=True, stop=True)
                    dpv = dp[:, 0:ng].rearrange("p (i j) -> p i j", j=BW)
                    dstm = denf[:, 1 + lo * 128 : 1 + hi * 128].rearrange(
                        "p (i j) -> p i j", j=128)
                    if t % 2 == 0:
                        nc.vector.tensor_copy(dstm, dpv[:, :, 1:129])
                        nc.vector.tensor_copy(dedge[:, lo:hi, :], dpv[:, :, 0:BW:129])
                    else:
                        nc.scalar.copy(dstm, dpv[:, :, 1:129])
                        nc.scalar.copy(dedge[:, lo:hi, :], dpv[:, :, 0:BW:129])

                def stitch_half(h):
                    if h == 0:
                        nc.vector.tensor_tensor(
                            denf[:, 129 : 129 + 16 * 128 : 128],
                            denf[:, 129 : 129 + 16 * 128 : 128],
                            dedge[:, 0:16, 1], op=ADD)
                        nc.vector.tensor_tensor(
                            denf[:, 128 : 128 + 16 * 128 : 128],
                            denf[:, 128 : 128 + 16 * 128 : 128],
                            dedge[:, 1:17, 0], op=ADD)
                        nc.vector.reciprocal_approx_fast(
                            denf[:, 0:2050], denf[:, 0:2050])
                    else:
                        nc.vector.tensor_tensor(
                            denf[:, 2177 : 2177 + 15 * 128 : 128],
                            denf[:, 2177 : 2177 + 15 * 128 : 128],
                            dedge[:, 16:31, 1], op=ADD)
                        nc.vector.tensor_tensor(
                            denf[:, 2176 : 2176 + 15 * 128 : 128],
                            denf[:, 2176 : 2176 + 15 * 128 : 128],
                            dedge[:, 17:32, 0], op=ADD)
                        nc.vector.reciprocal_approx_fast(
                            denf[:, 2050 : W + 2], denf[:, 2050 : W + 2])

                v_sb = vpool.tile([128, NBLK, 256], BF16, tag="v", name=f"v_sb{b}")
                e_sb = epool.tile([128, NBLK, BW], BF16, tag="e", name=f"e_sb{b}")
                s["v"], s["e"] = v_sb, e_sb
                vp = None
                gp = None
                for i in range(NBLK):
                    if i % 2 == 0:
                        vp = vppool.tile([128, 512], F32, tag="vp", name=f"vp{b}_{i}")
                    if i % 3 == 0:
                        gp = gppool.tile([128, 3 * BW], F32, tag="gp", name=f"gp{b}_{i}")
                    vslice = vp[:, (i % 2) * 256 : (i % 2) * 256 + 256]
                    gslice = gp[:, (i % 3) * BW : (i % 3) * BW + BW]
                    for kch in range(2):
                        xchunk = x_sb[:, kch, i * 128 : (i + 1) * 128]
                        nc.tensor.matmul(
                            vslice, xchunk, wv_sb[:, kch, :],
                            start=(kch == 0), stop=(kch == 1),
                        )
                        nc.tensor.matmul(
                            gslice, xchunk, y_sb[:, kch, i * 128 : i * 128 + BW],
                            start=(kch == 0), stop=(kch == 1),
                        )
                    if i % 2 == 1:
                        if i % 4 == 1:
                            nc.scalar.copy(v_sb[:, i - 1 : i + 1, :], vp[:])
                        else:
                            nc.vector.tensor_copy(v_sb[:, i - 1 : i + 1, :], vp[:])
                    if i % 3 == 2 or i == NBLK - 1:
                        g = i // 3
                        lo = g * 3
                        nb = i - lo + 1
                        ng = nb * BW
                        es = espool.tile([128, 3 * BW], BF16, tag="es",
                                         name=f"es{b}_{i}")
                        nc.scalar.activation(es[:, 0:ng], gp[:, 0:ng], Exp)
                        nc.gpsimd.tensor_tensor(
                            e_sb[:, lo : i + 1, :], es[:, 0:ng],
                            msk_sb[:, 0:nb, :], op=MUL,
                        )
                        if g >= 1:
                            den_group(g - 1)
                        if g == 6:
                            stitch_half(0)
                den_group(9)
                den_group(10)
                stitch_half(1)

            def phase_combine(b):
                s = st[b]
                v_sb, e_sb, recip = s["v"], s["e"], s["recip"]
                CB = [0, 12, 24, NBLK]          # chunk block ranges
                ofc = [
                    ofpool.tile([128, 2, (CB[c + 1] - CB[c]) * 128], F32,
                                tag="ofc", name=f"ofc{b}_{c}", bufs=3)
                    for c in range(3)
                ]
                oedge = egpool.tile([128, 2, NBLK, 2], F32, tag="oe", name=f"oe{b}")
                og = [None, None]

                def of_col(i):
                    c = i // 12
                    return c, (i - CB[c]) * 128

                def boundary_adds(i_lo, i_hi):
                    # left: of[i*128] += oedge[i-1, 1]; right: of[i*128-1] += oedge[i, 0]
                    segs = []
                    p = i_lo
                    while p < i_hi:
                        q = min(i_hi, ((p // 12) + 1) * 12)
                        segs.append((p, q))
                        p = q
                    for (a, z) in segs:
                        c = a // 12
                        base = CB[c] * 128
                        sc = a * 128 - base
                        ec = sc + (z - a - 1) * 128 + 1
                        for cch in range(2):
                            nc.gpsimd.tensor_tensor(
                                ofc[c][:, cch, sc : ec : 128],
                                ofc[c][:, cch, sc : ec : 128],
                                oedge[:, cch, a - 1 : z - 1, 1], op=ADD,
                            )
                    # rights target col i*128-1 (chunk of i-1)
                    segs = []
                    p = i_lo
                    while p < i_hi:
                        q = min(i_hi, (((p - 1) // 12) + 1) * 12 + 1)
                        segs.append((p, q))
                        p = q
                    for (a, z) in segs:
                        c = (a - 1) // 12
                        base = CB[c] * 128
                        sc = a * 128 - 1 - base
                        ec = sc + (z - a - 1) * 128 + 1
                        for cch in range(2):
                            nc.gpsimd.tensor_tensor(
                                ofc[c][:, cch, sc : ec : 128],
                                ofc[c][:, cch, sc : ec : 128],
                                oedge[:, cch, a : z, 0], op=ADD,
                            )

                def chunk_dma(c):
                    base = CB[c] * 128
                    ncols = (CB[c + 1] - CB[c]) * 128
                    for cch in range(2):
                        nc.sync.dma_start(
                            out_d[b, cch][:, base : base + ncols],
                            ofc[c][:, cch, :],
                        )

                for t in range(11):
                    lo = 3 * t
                    hi = min(lo + 3, NBLK)
                    nb = hi - lo
                    ng = nb * BW
                    eng = nc.vector if t % 2 == 0 else nc.gpsimd
                    for q in range(nb):
                        i = lo + q
                        eng.tensor_tensor(
                            e_sb[:, i, :], e_sb[:, i, :],
                            recip[:, i * 128 : i * 128 + BW], op=MUL)
                    og[0] = mmpool.tile([128, 512], F32, tag="mm", name=f"og0_{b}_{t}")
                    og[1] = mmpool.tile([128, 512], F32, tag="mm", name=f"og1_{b}_{t}")
                    for q in range(nb):
                        i = lo + q
                        for cch in range(2):
                            nc.tensor.matmul(
                                og[cch][:, q * BW : q * BW + BW],
                                v_sb[:, i, cch * 128 : (cch + 1) * 128],
                                e_sb[:, i, :],
                                start=True, stop=True,
                            )
                    c = lo // 12
                    base = CB[c] * 128
                    for cch in range(2):
                        ogv = og[cch][:, 0:ng].rearrange("p (i j) -> p i j", j=BW)
                        dst = ofc[c][:, cch, lo * 128 - base : hi * 128 - base]
                        dst = dst.rearrange("p (i j) -> p i j", j=128)
                        if cch == 0:
                            nc.scalar.copy(dst, ogv[:, :, 1:129])
                            nc.scalar.copy(oedge[:, cch, lo:hi, :], ogv[:, :, 0:BW:129])
                        else:
                            nc.vector.tensor_copy(dst, ogv[:, :, 1:129])
                            nc.vector.tensor_copy(
                                oedge[:, cch, lo:hi, :], ogv[:, :, 0:BW:129])
                    if t >= 1:
                        boundary_adds(max(3 * (t - 1), 1), 3 * t)
                    if t == 5:
                        chunk_dma(0)
                    if t == 9:
                        chunk_dma(1)
                boundary_adds(30, NBLK)
                chunk_dma(2)

            # software-pipelined emission across the two batches
            phase_load_y(0)
            phase_blocks(0)
            phase_load_y(1)
            phase_combine(0)
            phase_blocks(1)
            phase_combine(1)
    nc.compile()
    return nc


_GRAPH = {}


def kernel(x, Wq, bq, Wk, bk, Wv, bv):
    global _GRAPH
    x = np.asarray(x, np.float32)
    Wq = np.asarray(Wq, np.float32)
    Wk = np.asarray(Wk, np.float32)
    Wv = np.asarray(Wv, np.float32)
    bq = np.asarray(bq, np.float32)
    bk = np.asarray(bk, np.float32)
    bv = np.asarray(bv, np.float32)

    Mp = (Wq.T @ Wk) / 16.0                       # M'[c, c']
    ub = (Wk.T @ bq) / 16.0                       # per-c' bias on Y
    mask = np.zeros((128, BW), np.float32)
    for p in range(128):
        mask[p, p : p + 3] = 1.0
    onesm = np.ones((128, 128), np.float32)

    xs = x[:, :, 0, :]                            # [B, C, W]
    in_maps = []
    for core in range(NCORES):
        shard = xs[core * BL : (core + 1) * BL].reshape(BL, 2, 128, W)
        in_maps.append({
            "x": shard.astype(NPBF16),
            "Mp": Mp.reshape(2, 128, 256).astype(NPBF16),
            "WvT": Wv.T.reshape(2, 128, 256).astype(NPBF16),
            "ub": ub.reshape(2, 128, 1).astype(np.float32),
            "mask": mask.astype(NPBF16),
            "onesm": onesm.astype(NPBF16),
        })

    wb = bool(np.any(bq) or np.any(bk))
    if wb not in _GRAPH:
        _GRAPH[wb] = _build_graph(with_bias=wb)
    res = run_bass_kernel_spmd(_GRAPH[wb], in_maps, core_ids=list(range(NCORES)))
    outs = [np.asarray(r["out"], np.float32).reshape(BL, C, W) for r in res.results]
    full = np.concatenate(outs, axis=0)           # [B, C, W]
    full = full + bv[None, :, None]               # bias on V folds through softmax
    return full[:, :, None, :].astype(np.float32)


# revision 28
# speedup vs baseline: 468.5805x; 1.0047x over previous
import sys

sys.path.insert(0, "/opt/trn_rl_repo")

import numpy as np
import ml_dtypes

from concourse import bass, bacc, mybir
from concourse import tile
from concourse.bass_utils import run_bass_kernel_spmd

BF16 = mybir.dt.bfloat16
F32 = mybir.dt.float32
NPBF16 = ml_dtypes.bfloat16

B, C, H, W = 16, 256, 1, 4096
NCORES = 8
BL = B // NCORES          # batches per core
NBLK = W // 128           # 32 w-blocks of 128
BW = 130                  # band width per block (128 + 2 halo output cols)


def _build_graph(with_bias=False):
    """One NeuronCore graph (SPMD across 8 cores).

    Per batch b (2 per core), x_b = [C=256, W=4096] bf16 (CW layout):
      Y = M'^T x (CW, padded by one zero col each side), M' = Wq^T Wk / 16
      V = Wv x computed directly in WC orientation (x chunks as lhsT)
      per w-block i: gram G[p=w', j] = sum_c x[c, i*128+p] * Y[c, i*128-1+j]
      E = exp(G) * bandmask;  denom = colsum(E) via ones-matmul (replicated
      over partitions);  stitch cross-block denom pieces;  recip;  En = E*recip
      O_raw[c, j] = sum_p V[i*128+p, c] * En[p, j];  stitch + assemble; DMA out.
    """
    nc = bacc.Bacc(None, target_bir_lowering=False, debug=False)

    x_d = nc.declare_dram_parameter("x", [BL, 2, 128, W], BF16, isOutput=False)
    mp_d = nc.declare_dram_parameter("Mp", [2, 128, 256], BF16, isOutput=False)
    wv_d = nc.declare_dram_parameter("WvT", [2, 128, 256], BF16, isOutput=False)
    ub_d = nc.declare_dram_parameter("ub", [2, 128, 1], F32, isOutput=False)
    msk_d = nc.declare_dram_parameter("mask", [128, BW], BF16, isOutput=False)
    ones_d = nc.declare_dram_parameter("onesm", [128, 128], BF16, isOutput=False)
    out_d = nc.declare_dram_parameter("out", [BL, 2, 128, W], F32, isOutput=True)

    Exp = mybir.ActivationFunctionType.Exp
    Identity = mybir.ActivationFunctionType.Identity
    MUL = mybir.AluOpType.mult
    ADD = mybir.AluOpType.add

    with tile.TileContext(nc) as tc:
        with (
            tc.tile_pool(name="const", bufs=1) as cpool,
            tc.tile_pool(name="xin", bufs=2) as xpool,
            tc.tile_pool(name="ybuf", bufs=2) as ypool,
            tc.tile_pool(name="vbuf", bufs=2) as vpool,
            tc.tile_pool(name="ebuf", bufs=2) as epool,
            tc.tile_pool(name="escr", bufs=6) as espool,
            tc.tile_pool(name="den", bufs=2) as dpool,
            tc.tile_pool(name="edg", bufs=3) as egpool,
            tc.tile_pool(name="ofin", bufs=1) as ofpool,
            tc.tile_pool(name="mm2k", bufs=3, space=bass.MemorySpace.PSUM) as mmpool,
            tc.tile_pool(name="vps", bufs=2, space=bass.MemorySpace.PSUM) as vppool,
            tc.tile_pool(name="gps", bufs=3, space=bass.MemorySpace.PSUM) as gppool,
        ):
            # ---- constants ----
            mp_sb = cpool.tile([128, 2, 256], BF16, tag="mp")
            wv_sb = cpool.tile([128, 2, 256], BF16, tag="wv")
            ub_sb = cpool.tile([128, 2, 1], F32, tag="ub")
            msk_sb = cpool.tile([128, 3, BW], BF16, tag="msk")
            ones_sb = cpool.tile([128, 128], BF16, tag="ones")
            for ch in range(2):
                nc.scalar.dma_start(mp_sb[:, ch, :], mp_d[ch])
                nc.scalar.dma_start(wv_sb[:, ch, :], wv_d[ch])
                nc.scalar.dma_start(ub_sb[:, ch, :], ub_d[ch])
            for t in range(3):
                nc.scalar.dma_start(msk_sb[:, t, :], msk_d[:])
            nc.scalar.dma_start(ones_sb[:], ones_d[:])

            st = [dict() for _ in range(BL)]

            def phase_load_y(b):
                s = st[b]
                x_sb = xpool.tile([128, 2, W], BF16, tag="x", name=f"x_sb{b}")
                s["x"] = x_sb
                for (c0, c1) in [(0, 512), (512, W // 2), (W // 2, W)]:
                    for ch in range(2):
                        nc.sync.dma_start(
                            x_sb[:, ch, c0:c1],
                            x_d[b, ch][:, c0:c1],
                        )
                y_sb = ypool.tile([128, 2, W + 2], BF16, tag="y", name=f"y_sb{b}")
                s["y"] = y_sb
                for ch in range(2):
                    nc.vector.memset(y_sb[:, ch, 0:1], 0.0)
                    nc.vector.memset(y_sb[:, ch, W + 1 : W + 2], 0.0)
                for n in range(8):
                    for mch in range(2):
                        yp = mmpool.tile([128, 512], F32, tag="mm",
                                         name=f"yp{b}_{n}_{mch}")
                        for kch in range(2):
                            nc.tensor.matmul(
                                yp[:],
                                mp_sb[:, kch, mch * 128 : (mch + 1) * 128],
                                x_sb[:, kch, n * 512 : (n + 1) * 512],
                                start=(kch == 0),
                                stop=(kch == 1),
                            )
                        ydst = y_sb[:, mch, 1 + n * 512 : 1 + (n + 1) * 512]
                        if with_bias:
                            if mch == 0:
                                nc.scalar.activation(
                                    ydst, yp[:], Identity, bias=ub_sb[:, mch, :])
                            else:
                                nc.vector.tensor_scalar_add(
                                    ydst, yp[:], ub_sb[:, mch, :])
                        else:
                            if mch == 0:
                                nc.scalar.copy(ydst, yp[:])
                            else:
                                nc.vector.tensor_copy(ydst, yp[:])

            def phase_blocks(b):
                s = st[b]
                x_sb, y_sb = s["x"], s["y"]
                denf = dpool.tile([128, W + 2], F32, tag="df", name=f"denf{b}")
                dedge = egpool.tile([128, NBLK, 2], F32, tag="de", name=f"dedge{b}")
                recip = denf
                s["denf"], s["dedge"], s["recip"] = denf, dedge, recip
                nc.vector.memset(denf[:, 0:1], 1.0)
                nc.vector.memset(denf[:, W + 1 : W + 2], 1.0)

                def den_group(t):
                    lo = 3 * t
                    hi = min(lo + 3, NBLK)
                    nb = hi - lo
                    ng = nb * BW
                    dp = gppool.tile([128, 3 * BW], F32, tag="gp", name=f"dp{b}_{t}")
                    nc.tensor.matmul(dp[:, 0:ng], ones_sb[:], e_sb[:, lo:hi, :],
                                     start=True, stop=True)
                    dpv = dp[:, 0:ng].rearrange("p (i j) -> p i j", j=BW)
                    dstm = denf[:, 1 + lo * 128 : 1 + hi * 128].rearrange(
                        "p (i j) -> p i j", j=128)
                    if t % 2 == 0:
                        nc.vector.tensor_copy(dstm, dpv[:, :, 1:129])
                        nc.vector.tensor_copy(dedge[:, lo:hi, :], dpv[:, :, 0:BW:129])
                    else:
                        nc.scalar.copy(dstm, dpv[:, :, 1:129])
                        nc.scalar.copy(dedge[:, lo:hi, :], dpv[:, :, 0:BW:129])

                def stitch_half(h):
                    if h == 0:
                        nc.vector.tensor_tensor(
                            denf[:, 129 : 129 + 16 * 128 : 128],
                            denf[:, 129 : 129 + 16 * 128 : 128],
                            dedge[:, 0:16, 1], op=ADD)
                        nc.vector.tensor_tensor(
                            denf[:, 128 : 128 + 16 * 128 : 128],
                            denf[:, 128 : 128 + 16 * 128 : 128],
                            dedge[:, 1:17, 0], op=ADD)
                        nc.vector.reciprocal_approx_fast(
                            denf[:, 0:2050], denf[:, 0:2050])
                    else:
                        nc.vector.tensor_tensor(
                            denf[:, 2177 : 2177 + 15 * 128 : 128],
                            denf[:, 2177 : 2177 + 15 * 128 : 128],
                            dedge[:, 16:31, 1], op=ADD)
                        nc.vector.tensor_tensor(
                            denf[:, 2176 : 2176 + 15 * 128 : 128],
                            denf[:, 2176 : 2176 + 15 * 128 : 128],
                            dedge[:, 17:32, 0], op=ADD)
                        nc.vector.reciprocal_approx_fast(
                            denf[:, 2050 : W + 2], denf[:, 2050 : W + 2])

                v_sb = vpool.tile([128, NBLK, 256], BF16, tag="v", name=f"v_sb{b}")
                e_sb = epool.tile([128, NBLK, BW], BF16, tag="e", name=f"e_sb{b}")
                s["v"], s["e"] = v_sb, e_sb
                vp = None
                gp = None
                for i in range(NBLK):
                    if i % 2 == 0:
                        vp = vppool.tile([128, 512], F32, tag="vp", name=f"vp{b}_{i}")
                    if i % 3 == 0:
                        gp = gppool.tile([128, 3 * BW], F32, tag="gp", name=f"gp{b}_{i}")
                    vslice = vp[:, (i % 2) * 256 : (i % 2) * 256 + 256]
                    gslice = gp[:, (i % 3) * BW : (i % 3) * BW + BW]
                    for kch in range(2):
                        xchunk = x_sb[:, kch, i * 128 : (i + 1) * 128]
                        nc.tensor.matmul(
                            vslice, xchunk, wv_sb[:, kch, :],
                            start=(kch == 0), stop=(kch == 1),
                        )
                        nc.tensor.matmul(
                            gslice, xchunk, y_sb[:, kch, i * 128 : i * 128 + BW],
                            start=(kch == 0), stop=(kch == 1),
                        )
                    if i % 2 == 1:
                        if i % 4 == 1:
                            nc.scalar.copy(v_sb[:, i - 1 : i + 1, :], vp[:])
                        else:
                            nc.vector.tensor_copy(v_sb[:, i - 1 : i + 1, :], vp[:])
                    if i % 3 == 2 or i == NBLK - 1:
                        g = i // 3
                        lo = g * 3
                        nb = i - lo + 1
                        ng = nb * BW
                        es = espool.tile([128, 3 * BW], BF16, tag="es",
                                         name=f"es{b}_{i}")
                        nc.scalar.activation(es[:, 0:ng], gp[:, 0:ng], Exp)
                        nc.gpsimd.tensor_tensor(
                            e_sb[:, lo : i + 1, :], es[:, 0:ng],
                            msk_sb[:, 0:nb, :], op=MUL,
                        )
                        if g >= 1:
                            den_group(g - 1)
                        if g == 6:
                            stitch_half(0)
                den_group(9)
                den_group(10)
                stitch_half(1)

            def phase_combine(b):
                s = st[b]
                v_sb, e_sb, recip = s["v"], s["e"], s["recip"]
                CB = [0, 12, 24, NBLK]          # chunk block ranges
                ofc = [
                    ofpool.tile([128, 2, (CB[c + 1] - CB[c]) * 128], F32,
                                tag="ofc", name=f"ofc{b}_{c}", bufs=3)
                    for c in range(3)
                ]
                oedge = egpool.tile([128, 2, NBLK, 2], F32, tag="oe", name=f"oe{b}")
                og = [None, None]

                def of_col(i):
                    c = i // 12
                    return c, (i - CB[c]) * 128

                def boundary_adds(i_lo, i_hi):
                    # left: of[i*128] += oedge[i-1, 1]; right: of[i*128-1] += oedge[i, 0]
                    segs = []
                    p = i_lo
                    while p < i_hi:
                        q = min(i_hi, ((p // 12) + 1) * 12)
                        segs.append((p, q))
                        p = q
                    for (a, z) in segs:
                        c = a // 12
                        base = CB[c] * 128
                        sc = a * 128 - base
                        ec = sc + (z - a - 1) * 128 + 1
                        for cch in range(2):
                            nc.gpsimd.tensor_tensor(
                                ofc[c][:, cch, sc : ec : 128],
                                ofc[c][:, cch, sc : ec : 128],
                                oedge[:, cch, a - 1 : z - 1, 1], op=ADD,
                            )
                    # rights target col i*128-1 (chunk of i-1)
                    segs = []
                    p = i_lo
                    while p < i_hi:
                        q = min(i_hi, (((p - 1) // 12) + 1) * 12 + 1)
                        segs.append((p, q))
                        p = q
                    for (a, z) in segs:
                        c = (a - 1) // 12
                        base = CB[c] * 128
                        sc = a * 128 - 1 - base
                        ec = sc + (z - a - 1) * 128 + 1
                        for cch in range(2):
                            nc.gpsimd.tensor_tensor(
                                ofc[c][:, cch, sc : ec : 128],
                                ofc[c][:, cch, sc : ec : 128],
                                oedge[:, cch, a : z, 0], op=ADD,
                            )

                def chunk_dma(c):
                    base = CB[c] * 128
                    ncols = (CB[c + 1] - CB[c]) * 128
                    for cch in range(2):
                        nc.sync.dma_start(
                            out_d[b, cch][:, base : base + ncols],
                            ofc[c][:, cch, :],
                        )

                for t in range(11):
                    lo = 3 * t
                    hi = min(lo + 3, NBLK)
                    nb = hi - lo
                    ng = nb * BW
                    eng = nc.vector if t % 2 == 0 else nc.gpsimd
                    for q in range(nb):
                        i = lo + q
                        eng.tensor_tensor(
                            e_sb[:, i, :], e_sb[:, i, :],
                            recip[:, i * 128 : i * 128 + BW], op=MUL)
                    og[0] = mmpool.tile([128, 512], F32, tag="mm", name=f"og0_{b}_{t}")
                    og[1] = mmpool.tile([128, 512], F32, tag="mm", name=f"og1_{b}_{t}")
                    for q in range(nb):
                        i = lo + q
                        for cch in range(2):
                            nc.tensor.matmul(
                                og[cch][:, q * BW : q * BW + BW],
                                v_sb[:, i, cch * 128 : (cch + 1) * 128],
                                e_sb[:, i, :],
                                start=True, stop=True,
                            )
                    c = lo // 12
                    base = CB[c] * 128
                    for cch in range(2):
                        ogv = og[cch][:, 0:ng].rearrange("p (i j) -> p i j", j=BW)
                        dst = ofc[c][:, cch, lo * 128 - base : hi * 128 - base]
                        dst = dst.rearrange("p (i j) -> p i j", j=128)
                        if cch == 0:
                            nc.scalar.copy(dst, ogv[:, :, 1:129])
                            nc.scalar.copy(oedge[:, cch, lo:hi, :], ogv[:, :, 0:BW:129])
                        else:
                            nc.vector.tensor_copy(dst, ogv[:, :, 1:129])
                            nc.vector.tensor_copy(
                                oedge[:, cch, lo:hi, :], ogv[:, :, 0:BW:129])
                    if t >= 1:
                        boundary_adds(max(3 * (t - 1), 1), 3 * t)
                    if t == 5:
                        chunk_dma(0)
                    if t == 9:
                        chunk_dma(1)
                boundary_adds(30, NBLK)
                chunk_dma(2)

            # software-pipelined emission across the two batches
            phase_load_y(0)
            phase_blocks(0)
            phase_load_y(1)
            phase_combine(0)
            phase_blocks(1)
            phase_combine(1)
    nc.compile()
    return nc


_GRAPH = {}


def kernel(x, Wq, bq, Wk, bk, Wv, bv):
    global _GRAPH
    x = np.asarray(x, np.float32)
    Wq = np.asarray(Wq, np.float32)
    Wk = np.asarray(Wk, np.float32)
    Wv = np.asarray(Wv, np.float32)
    bq = np.asarray(bq, np.float32)
    bk = np.asarray(bk, np.float32)
    bv = np.asarray(bv, np.float32)

    Mp = (Wq.T @ Wk) / 16.0                       # M'[c, c']
    ub = (Wk.T @ bq) / 16.0                       # per-c' bias on Y
    mask = np.zeros((128, BW), np.float32)
    for p in range(128):
        mask[p, p : p + 3] = 1.0
    onesm = np.ones((128, 128), np.float32)

    xs = x[:, :, 0, :]                            # [B, C, W]
    in_maps = []
    for core in range(NCORES):
        shard = xs[core * BL : (core + 1) * BL].reshape(BL, 2, 128, W)
        in_maps.append({
            "x": shard.astype(NPBF16),
            "Mp": Mp.reshape(2, 128, 256).astype(NPBF16),
            "WvT": Wv.T.reshape(2, 128, 256).astype(NPBF16),
            "ub": ub.reshape(2, 128, 1).astype(np.float32),
            "mask": mask.astype(NPBF16),
            "onesm": onesm.astype(NPBF16),
        })

    wb = bool(np.any(bq) or np.any(bk))
    if wb not in _GRAPH:
        _GRAPH[wb] = _build_graph(with_bias=wb)
    res = run_bass_kernel_spmd(_GRAPH[wb], in_maps, core_ids=list(range(NCORES)))
    outs = [np.asarray(r["out"], np.float32).reshape(BL, C, W) for r in res.results]
    full = np.concatenate(outs, axis=0)           # [B, C, W]
    full = full + bv[None, :, None]               # bias on V folds through softmax
    return full[:, :, None, :].astype(np.float32)


# revision 37
# speedup vs baseline: 504.5853x; 1.0768x over previous
import sys

sys.path.insert(0, "/opt/trn_rl_repo")

import numpy as np
import ml_dtypes

from concourse import bass, bacc, mybir
from concourse import tile
from concourse.bass_utils import run_bass_kernel_spmd

BF16 = mybir.dt.bfloat16
F32 = mybir.dt.float32
NPBF16 = ml_dtypes.bfloat16

B, C, H, W = 16, 256, 1, 4096
NCORES = 8
BL = B // NCORES          # batches per core
NBLK = W // 128           # 32 w-blocks of 128
BW = 130                  # band width per block (128 + 2 halo output cols)


def _build_graph(with_bias=False):
    """One NeuronCore graph (SPMD across 8 cores).

    Per batch b (2 per core), x_b = [C=256, W=4096] bf16 (CW layout):
      Y = M'^T x (CW, padded by one zero col each side), M' = Wq^T Wk / 16
      V = Wv x computed directly in WC orientation (x chunks as lhsT)
      per w-block i: gram G[p=w', j] = sum_c x[c, i*128+p] * Y[c, i*128-1+j]
      E = exp(G) * bandmask;  denom = colsum(E) via ones-matmul (replicated
      over partitions);  stitch cross-block denom pieces;  recip;  En = E*recip
      O_raw[c, j] = sum_p V[i*128+p, c] * En[p, j];  stitch + assemble; DMA out.
    """
    nc = bacc.Bacc(None, target_bir_lowering=False, debug=False)

    x_d = nc.declare_dram_parameter("x", [BL, 2, 128, W], BF16, isOutput=False)
    mp_d = nc.declare_dram_parameter("Mp", [2, 128, 256], BF16, isOutput=False)
    wv_d = nc.declare_dram_parameter("WvT", [2, 128, 256], BF16, isOutput=False)
    ub_d = nc.declare_dram_parameter("ub", [2, 128, 1], F32, isOutput=False)
    msk_d = nc.declare_dram_parameter("mask", [128, BW], BF16, isOutput=False)
    ones_d = nc.declare_dram_parameter("onesm", [128, 128], BF16, isOutput=False)
    out_d = nc.declare_dram_parameter("out", [BL, 2, 128, W], F32, isOutput=True)

    Exp = mybir.ActivationFunctionType.Exp
    Identity = mybir.ActivationFunctionType.Identity
    MUL = mybir.AluOpType.mult
    ADD = mybir.AluOpType.add

    with tile.TileContext(nc) as tc:
        with (
            tc.tile_pool(name="const", bufs=1) as cpool,
            tc.tile_pool(name="xin", bufs=2) as xpool,
            tc.tile_pool(name="ybuf", bufs=2) as ypool,
            tc.tile_pool(name="vbuf", bufs=2) as vpool,
            tc.tile_pool(name="ebuf", bufs=2) as epool,
            tc.tile_pool(name="escr", bufs=6) as espool,
            tc.tile_pool(name="den", bufs=2) as dpool,
            tc.tile_pool(name="edg", bufs=3) as egpool,
            tc.tile_pool(name="ofin", bufs=1) as ofpool,
            tc.tile_pool(name="mm2k", bufs=3, space=bass.MemorySpace.PSUM) as mmpool,
            tc.tile_pool(name="vps", bufs=2, space=bass.MemorySpace.PSUM) as vppool,
            tc.tile_pool(name="gps", bufs=3, space=bass.MemorySpace.PSUM) as gppool,
        ):
            # ---- constants ----
            mp_sb = cpool.tile([128, 2, 256], BF16, tag="mp")
            wv_sb = cpool.tile([128, 2, 256], BF16, tag="wv")
            ub_sb = cpool.tile([128, 2, 1], F32, tag="ub")
            msk_sb = cpool.tile([128, 3, BW], BF16, tag="msk")
            ones_sb = cpool.tile([128, 128], BF16, tag="ones")
            for ch in range(2):
                nc.scalar.dma_start(mp_sb[:, ch, :], mp_d[ch])
                nc.scalar.dma_start(wv_sb[:, ch, :], wv_d[ch])
                nc.scalar.dma_start(ub_sb[:, ch, :], ub_d[ch])
            for t in range(3):
                nc.scalar.dma_start(msk_sb[:, t, :], msk_d[:])
            nc.scalar.dma_start(ones_sb[:], ones_d[:])

            st = [dict() for _ in range(BL)]

            def phase_load_y(b):
                s = st[b]
                x_sb = xpool.tile([128, 2, W], BF16, tag="x", name=f"x_sb{b}")
                s["x"] = x_sb
                for (c0, c1) in [(0, 512), (512, W // 2), (W // 2, W)]:
                    for ch in range(2):
                        nc.sync.dma_start(
                            x_sb[:, ch, c0:c1],
                            x_d[b, ch][:, c0:c1],
                        )
                y_sb = ypool.tile([128, 2, W + 2], BF16, tag="y", name=f"y_sb{b}")
                s["y"] = y_sb
                for ch in range(2):
                    nc.vector.memset(y_sb[:, ch, 0:1], 0.0)
                    nc.vector.memset(y_sb[:, ch, W + 1 : W + 2], 0.0)
                for n in range(8):
                    for mch in range(2):
                        yp = mmpool.tile([128, 512], F32, tag="mm",
                                         name=f"yp{b}_{n}_{mch}")
                        for kch in range(2):
                            nc.tensor.matmul(
                                yp[:],
                                mp_sb[:, kch, mch * 128 : (mch + 1) * 128],
                                x_sb[:, kch, n * 512 : (n + 1) * 512],
                                start=(kch == 0),
                                stop=(kch == 1),
                            )
                        ydst = y_sb[:, mch, 1 + n * 512 : 1 + (n + 1) * 512]
                        if with_bias:
                            if mch == 0:
                                nc.scalar.activation(
                                    ydst, yp[:], Identity, bias=ub_sb[:, mch, :])
                            else:
                                nc.vector.tensor_scalar_add(
                                    ydst, yp[:], ub_sb[:, mch, :])
                        else:
                            if mch == 0:
                                nc.scalar.copy(ydst, yp[:])
                            else:
                                nc.vector.tensor_copy(ydst, yp[:])

            def phase_blocks(b):
                s = st[b]
                x_sb, y_sb = s["x"], s["y"]
                denf = dpool.tile([128, W + 2], F32, tag="df", name=f"denf{b}")
                dedge = egpool.tile([128, NBLK, 2], F32, tag="de", name=f"dedge{b}")
                recip = denf
                s["denf"], s["dedge"], s["recip"] = denf, dedge, recip
                nc.vector.memset(denf[:, 0:1], 1.0)
                nc.vector.memset(denf[:, W + 1 : W + 2], 1.0)

                def den_group(t):
                    lo = 3 * t
                    hi = min(lo + 3, NBLK)
                    nb = hi - lo
                    ng = nb * BW
                    dp = gppool.tile([128, 3 * BW], F32, tag="gp", name=f"dp{b}_{t}")
                    nc.tensor.matmul(dp[:, 0:ng], ones_sb[:], e_sb[:, lo:hi, :],
                                     start=True, stop=True)
                    dpv = dp[:, 0:ng].rearrange("p (i j) -> p i j", j=BW)
                    dstm = denf[:, 1 + lo * 128 : 1 + hi * 128].rearrange(
                        "p (i j) -> p i j", j=128)
                    if t % 2 == 0:
                        nc.vector.tensor_copy(dstm, dpv[:, :, 1:129])
                        nc.vector.tensor_copy(dedge[:, lo:hi, :], dpv[:, :, 0:BW:129])
                    else:
                        nc.scalar.copy(dstm, dpv[:, :, 1:129])
                        nc.scalar.copy(dedge[:, lo:hi, :], dpv[:, :, 0:BW:129])

                def stitch_half(h):
                    if h == 0:
                        nc.vector.tensor_tensor(
                            denf[:, 129 : 129 + 16 * 128 : 128],
                            denf[:, 129 : 129 + 16 * 128 : 128],
                            dedge[:, 0:16, 1], op=ADD)
                        nc.vector.tensor_tensor(
                            denf[:, 128 : 128 + 16 * 128 : 128],
                            denf[:, 128 : 128 + 16 * 128 : 128],
                            dedge[:, 1:17, 0], op=ADD)
                        nc.vector.reciprocal_approx_fast(
                            denf[:, 0:2050], denf[:, 0:2050])
                    else:
                        nc.vector.tensor_tensor(
                            denf[:, 2177 : 2177 + 15 * 128 : 128],
                            denf[:, 2177 : 2177 + 15 * 128 : 128],
                            dedge[:, 16:31, 1], op=ADD)
                        nc.vector.tensor_tensor(
                            denf[:, 2176 : 2176 + 15 * 128 : 128],
                            denf[:, 2176 : 2176 + 15 * 128 : 128],
                            dedge[:, 17:32, 0], op=ADD)
                        nc.vector.reciprocal_approx_fast(
                            denf[:, 2050 : W + 2], denf[:, 2050 : W + 2])

                v_sb = vpool.tile([128, NBLK, 256], BF16, tag="v", name=f"v_sb{b}")
                e_sb = epool.tile([128, NBLK, BW], BF16, tag="e", name=f"e_sb{b}")
                s["v"], s["e"] = v_sb, e_sb
                vp = None
                gp = None
                for i in range(NBLK):
                    if i % 2 == 0:
                        vp = vppool.tile([128, 512], F32, tag="vp", name=f"vp{b}_{i}")
                    if i % 3 == 0:
                        gp = gppool.tile([128, 3 * BW], F32, tag="gp", name=f"gp{b}_{i}")
                    vslice = vp[:, (i % 2) * 256 : (i % 2) * 256 + 256]
                    gslice = gp[:, (i % 3) * BW : (i % 3) * BW + BW]
                    for kch in range(2):
                        xchunk = x_sb[:, kch, i * 128 : (i + 1) * 128]
                        nc.tensor.matmul(
                            vslice, xchunk, wv_sb[:, kch, :],
                            start=(kch == 0), stop=(kch == 1),
                        )
                        nc.tensor.matmul(
                            gslice, xchunk, y_sb[:, kch, i * 128 : i * 128 + BW],
                            start=(kch == 0), stop=(kch == 1),
                        )
                    if i % 2 == 1:
                        if i % 4 == 1:
                            nc.scalar.copy(v_sb[:, i - 1 : i + 1, :], vp[:])
                        else:
                            nc.vector.tensor_copy(v_sb[:, i - 1 : i + 1, :], vp[:])
                    if i % 3 == 2 or i == NBLK - 1:
                        g = i // 3
                        lo = g * 3
                        nb = i - lo + 1
                        ng = nb * BW
                        es = espool.tile([128, 3 * BW], BF16, tag="es",
                                         name=f"es{b}_{i}")
                        nc.scalar.activation(es[:, 0:ng], gp[:, 0:ng], Exp)
                        nc.gpsimd.tensor_tensor(
                            e_sb[:, lo : i + 1, :], es[:, 0:ng],
                            msk_sb[:, 0:nb, :], op=MUL,
                        )
                        if g >= 1:
                            den_group(g - 1)
                        if g == 6:
                            stitch_half(0)
                den_group(9)
                den_group(10)
                stitch_half(1)

            def phase_combine(b):
                s = st[b]
                v_sb, e_sb, recip = s["v"], s["e"], s["recip"]
                CB = [0, 12, 24, NBLK]          # chunk block ranges
                ofc = [
                    ofpool.tile([128, 2, (CB[c + 1] - CB[c]) * 128], F32,
                                tag="ofc", name=f"ofc{b}_{c}", bufs=3)
                    for c in range(3)
                ]
                oedge = egpool.tile([128, 2, NBLK, 2], F32, tag="oe", name=f"oe{b}")
                og = [None, None]

                def of_col(i):
                    c = i // 12
                    return c, (i - CB[c]) * 128

                def boundary_adds(i_lo, i_hi):
                    # left: of[i*128] += oedge[i-1, 1]; right: of[i*128-1] += oedge[i, 0]
                    segs = []
                    p = i_lo
                    while p < i_hi:
                        q = min(i_hi, ((p // 12) + 1) * 12)
                        segs.append((p, q))
                        p = q
                    for (a, z) in segs:
                        c = a // 12
                        base = CB[c] * 128
                        sc = a * 128 - base
                        ec = sc + (z - a - 1) * 128 + 1
                        for cch in range(2):
                            nc.gpsimd.tensor_tensor(
                                ofc[c][:, cch, sc : ec : 128],
                                ofc[c][:, cch, sc : ec : 128],
                                oedge[:, cch, a - 1 : z - 1, 1], op=ADD,
                            )
                    # rights target col i*128-1 (chunk of i-1)
                    segs = []
                    p = i_lo
                    while p < i_hi:
                        q = min(i_hi, (((p - 1) // 12) + 1) * 12 + 1)
                        segs.append((p, q))
                        p = q
                    for (a, z) in segs:
                        c = (a - 1) // 12
                        base = CB[c] * 128
                        sc = a * 128 - 1 - base
                        ec = sc + (z - a - 1) * 128 + 1
                        for cch in range(2):
                            nc.gpsimd.tensor_tensor(
                                ofc[c][:, cch, sc : ec : 128],
                                ofc[c][:, cch, sc : ec : 128],
                                oedge[:, cch, a : z, 0], op=ADD,
                            )

                def chunk_dma(c):
                    base = CB[c] * 128
                    ncols = (CB[c + 1] - CB[c]) * 128
                    for cch in range(2):
                        nc.sync.dma_start(
                            out_d[b, cch][:, base : base + ncols],
                            ofc[c][:, cch, :],
                        )

                for t in range(11):
                    lo = 3 * t
                    hi = min(lo + 3, NBLK)
                    nb = hi - lo
                    ng = nb * BW
                    eng = nc.gpsimd
                    for q in range(nb):
                        i = lo + q
                        eng.tensor_tensor(
                            e_sb[:, i, :], e_sb[:, i, :],
                            recip[:, i * 128 : i * 128 + BW], op=MUL)
                    og[0] = mmpool.tile([128, 512], F32, tag="mm", name=f"og0_{b}_{t}")
                    og[1] = mmpool.tile([128, 512], F32, tag="mm", name=f"og1_{b}_{t}")
                    for q in range(nb):
                        i = lo + q
                        for cch in range(2):
                            nc.tensor.matmul(
                                og[cch][:, q * BW : q * BW + BW],
                                v_sb[:, i, cch * 128 : (cch + 1) * 128],
                                e_sb[:, i, :],
                                start=True, stop=True,
                            )
                    c = lo // 12
                    base = CB[c] * 128
                    for cch in range(2):
                        ogv = og[cch][:, 0:ng].rearrange("p (i j) -> p i j", j=BW)
                        dst = ofc[c][:, cch, lo * 128 - base : hi * 128 - base]
                        dst = dst.rearrange("p (i j) -> p i j", j=128)
                        if cch == 0:
                            nc.scalar.copy(dst, ogv[:, :, 1:129])
                            nc.scalar.copy(oedge[:, cch, lo:hi, :], ogv[:, :, 0:BW:129])
                        else:
                            nc.vector.tensor_copy(dst, ogv[:, :, 1:129])
                            nc.vector.tensor_copy(
                                oedge[:, cch, lo:hi, :], ogv[:, :, 0:BW:129])
                    if t >= 1:
                        boundary_adds(max(3 * (t - 1), 1), 3 * t)
                    if t == 5:
                        chunk_dma(0)
                    if t == 9:
                        chunk_dma(1)
                boundary_adds(30, NBLK)
                chunk_dma(2)

            # software-pipelined emission across the two batches
            phase_load_y(0)
            phase_blocks(0)
            phase_load_y(1)
            phase_combine(0)
            phase_blocks(1)
            phase_combine(1)
    nc.compile()
    return nc


_GRAPH = {}


def kernel(x, Wq, bq, Wk, bk, Wv, bv):
    global _GRAPH
    x = np.asarray(x, np.float32)
    Wq = np.asarray(Wq, np.float32)
    Wk = np.asarray(Wk, np.float32)
    Wv = np.asarray(Wv, np.float32)
    bq = np.asarray(bq, np.float32)
    bk = np.asarray(bk, np.float32)
    bv = np.asarray(bv, np.float32)

    Mp = (Wq.T @ Wk) / 16.0                       # M'[c, c']
    ub = (Wk.T @ bq) / 16.0                       # per-c' bias on Y
    mask = np.zeros((128, BW), np.float32)
    for p in range(128):
        mask[p, p : p + 3] = 1.0
    onesm = np.ones((128, 128), np.float32)

    xs = x[:, :, 0, :]                            # [B, C, W]
    in_maps = []
    for core in range(NCORES):
        shard = xs[core * BL : (core + 1) * BL].reshape(BL, 2, 128, W)
        in_maps.append({
            "x": shard.astype(NPBF16),
            "Mp": Mp.reshape(2, 128, 256).astype(NPBF16),
            "WvT": Wv.T.reshape(2, 128, 256).astype(NPBF16),
            "ub": ub.reshape(2, 128, 1).astype(np.float32),
            "mask": mask.astype(NPBF16),
            "onesm": onesm.astype(NPBF16),
        })

    wb = bool(np.any(bq) or np.any(bk))
    if wb not in _GRAPH:
        _GRAPH[wb] = _build_graph(with_bias=wb)
    res = run_bass_kernel_spmd(_GRAPH[wb], in_maps, core_ids=list(range(NCORES)))
    outs = [np.asarray(r["out"], np.float32).reshape(BL, C, W) for r in res.results]
    full = np.concatenate(outs, axis=0)           # [B, C, W]
    full = full + bv[None, :, None]               # bias on V folds through softmax
    return full[:, :, None, :].astype(np.float32)
